# revision 30
# baseline (speedup 1.0000x reference)
"""Trainium2 Bass kernel for nn_CAM_Module (channel-attention module).

Math per batch n (N = B*D = 128 independent problems):
    V = x[b, :, d, :, :].reshape(C, S)          # C=128, S=4096
    G = V @ V.T                                  # (C, C) Gram / energy
    A = softmax(-G) row-wise (stabilized with rowmin subtract)
    out_n = (gamma * A + I) @ V                  # == gamma*(A@V) + V

Sharding: data-parallel over n across 8 NeuronCores (16 n per core).

The kernel is HBM-bound: per core 16 problems x (1 MB in + 1 MB out) in
fp16 ~= 32 MB at ~350 GB/s ~= 92 us. Design notes:
  - I/O in fp16 (host converts): halves DMA vs fp32, fp16 PE transposes
    run 1 cyc/row (fp32 costs 2), and fp16 gram/stage-2 matmuls run at
    full PE rate.
  - The stage-2 matmul uses M^T = (gamma*A + I)^T built on-chip, so
    out = M^T.T @ V needs NO elementwise epilogue: the stage-2 PSUM
    result is final; ACT/DVE just cast-copy it to fp16 (3/1 split,
    [C,1024] granularity to amortize per-op overhead).
  - U = V^T via 32 PE transposes; U chunks reach SBUF in 4 [C,1024]
    DVE copies (fp16 2x mode).
  - Software pipeline 2 deep: softmax tail (numer^T transpose + +I) of
    problem n runs early in iteration n+1, stage 2 of n in iteration
    n+2, stage-2 units interleaved between transpose/gram batches so
    PSUM recycling never stalls the PE and the PE p-state stays high.
"""

import numpy as np
from contextlib import ExitStack
from types import SimpleNamespace

import concourse.bass as bass
import concourse.tile as tile
from concourse import bacc, mybir
from concourse.bass_utils import run_bass_kernel_spmd

B, C, D, H, W = 4, 128, 32, 64, 64
S = H * W                  # 4096
N_TOTAL = B * D            # 128
N_CORES = 8
N_PER_CORE = N_TOTAL // N_CORES   # 16

FP = mybir.dt.float32
FP16 = mybir.dt.float16
AF = mybir.ActivationFunctionType
AX = mybir.AxisListType
OP = mybir.AluOpType

_CACHE = {}


def build_program(n_per_core=N_PER_CORE):
    key = n_per_core
    if key in _CACHE:
        return _CACHE[key]

    nc = bacc.Bacc(
        "TRN2", target_bir_lowering=False, debug=False, num_devices=N_CORES
    )
    xs = nc.dram_tensor("xs", [n_per_core, C, S], FP16, kind="ExternalInput").ap()
    gamma_b = nc.dram_tensor("gamma_b", [C, 1], FP, kind="ExternalInput").ap()
    ident = nc.dram_tensor("ident", [C, C], FP16, kind="ExternalInput").ap()
    out = nc.dram_tensor("out", [n_per_core, C, S], FP16, kind="ExternalOutput").ap()

    NCHUNK = S // C            # 32 transpose chunks per n
    NTB = 4                    # transpose/copy batches per n ([C,1024] each)
    NJ = S // 512              # 8 single-matmul stage-2 units per n
    PIPE = 2                   # stage-2 depth: st2(n) emitted in iter n+PIPE
    LOOK = 3                   # V-load lookahead: iter n issues v(n+LOOK) DMAs

    with tile.TileContext(nc) as tc, ExitStack() as ctx:
        const_pool = ctx.enter_context(tc.tile_pool(name="const", bufs=1))
        v_pool = ctx.enter_context(tc.tile_pool(name="v", bufs=PIPE + LOOK + 2))
        u_pool = ctx.enter_context(tc.tile_pool(name="u", bufs=2))
        small_pool = ctx.enter_context(tc.tile_pool(name="small", bufs=PIPE + 2))
        osb_pool = ctx.enter_context(tc.tile_pool(name="osb", bufs=2))
        tr_ps_pool = ctx.enter_context(tc.tile_pool(name="trps", bufs=2, space="PSUM"))
        g_ps_pool = ctx.enter_context(tc.tile_pool(name="gps", bufs=1, space="PSUM"))
        at_ps_pool = ctx.enter_context(tc.tile_pool(name="atps", bufs=1, space="PSUM"))
        o_ps_pool = ctx.enter_context(tc.tile_pool(name="ops", bufs=2, space="PSUM"))

        id_sb = const_pool.tile([C, C], FP16)
        nc.sync.dma_start(id_sb[:], ident[:])
        gam_sb = const_pool.tile([C, 1], FP)

        vmap = {}

        def load_v(m):
            if not (0 <= m < n_per_core) or m in vmap:
                return
            t = v_pool.tile([C, S], FP16, tag="v_sb")
            for h in range(4):
                nc.sync.dma_start(
                    t[:, 1024 * h : 1024 * (h + 1)],
                    xs[m, :, 1024 * h : 1024 * (h + 1)],
                )
            vmap[m] = t

        sm = {}        # n -> per-problem context
        pend_at = []   # n's whose softmax tail (transpose + +I) is pending
        pend_s2 = []   # stage-2 unit lists pending emission

        def at_finish(pc):
            # abt_eff[m, c] = gamma * A[c, m] + I[m, c]  (stage-2 stationary)
            at_ps = at_ps_pool.tile([C, C], FP16, tag="at_ps")
            nc.tensor.transpose(at_ps[:], pc.nsc[:], id_sb[:])
            abt = small_pool.tile([C, C], FP16, tag="abt")
            nc.vector.tensor_tensor(abt[:], at_ps[:], id_sb[:], op=OP.add)
            pc.abt_eff = abt

        def copy_o(eng, dst, src):
            if eng is nc.scalar:
                eng.copy(dst, src)
            else:
                eng.tensor_copy(dst, src)

        def make_stage2_units(pc, drain=False):
            # out_n = abt_eff.T @ V in 4 [C,1024] units (2 matmuls each into
            # one 2-bank PSUM tile); the PSUM values are final. The exp of
            # the previous problem occupies ~1.2us at the head of ACT's
            # per-iteration queue, so two of the four cast-copies lend their
            # second half to the DVE to keep ACT's tail ahead of the PSUM
            # recycle. Halves leave via gpsimd.
            st = SimpleNamespace(o_sb=None)

            def unit(u):
                def f():
                    if st.o_sb is None:
                        st.o_sb = osb_pool.tile([C, S], FP16, tag="o_sb")
                    o_ps = o_ps_pool.tile([C, 1024], FP, tag="o_ps")
                    for h in range(2):
                        j = 2 * u + h
                        nc.tensor.matmul(
                            o_ps[:, 512 * h : 512 * (h + 1)],
                            pc.abt_eff[:],
                            pc.v_sb[:, 512 * j : 512 * (j + 1)],
                            start=True, stop=True,
                        )
                    dst = st.o_sb[:, 1024 * u : 1024 * (u + 1)]
                    if drain:
                        # drain is PE-paced: split every copy across both
                        # engines so o_ps recycles as fast as possible
                        copy_o(nc.vector, dst[:, :512], o_ps[:, :512])
                        copy_o(nc.scalar, dst[:, 512:], o_ps[:, 512:])
                    elif u == 3:
                        # the last unit's copy gates the PSUM slot for the
                        # next problem; split it, with the DVE half emitted
                        # late so it doesn't displace the u-copies that
                        # feed the gram matmuls
                        copy_o(nc.scalar, dst[:, :512], o_ps[:, :512])

                        def late():
                            copy_o(nc.vector, dst[:, 512:], o_ps[:, 512:])
                            nc.gpsimd.dma_start(
                                out[pc.n, :, 2048:], st.o_sb[:, 2048:]
                            )

                        pc.late = late
                    else:
                        copy_o(nc.scalar, dst, o_ps[:])
                    if u == 1:
                        nc.gpsimd.dma_start(
                            out[pc.n, :, :2048], st.o_sb[:, :2048]
                        )
                    elif u == 3 and drain:
                        nc.gpsimd.dma_start(
                            out[pc.n, :, 2048:], st.o_sb[:, 2048:]
                        )
                return f

            units = [unit(u) for u in range(4)]
            if not drain:
                units.append(lambda: pc.late())
            return units

        # Only v(0) is prefetched up front: the first transposes need just
        # its first chunk, and issuing all LOOK loads here would serialize
        # ~9 us of DMA on the sync ring before the PE can start.  The
        # lookahead window fills during the first iterations instead.
        # gamma isn't needed until the end of iteration 0, so its DMA goes
        # behind v(0) on the sync ring.
        load_v(0)
        nc.sync.dma_start(gam_sb[:], gamma_b[:])

        for n in range(n_per_core + PIPE):
            if n < n_per_core:
                load_v(n + 1)
                load_v(n + LOOK)
                pc = SimpleNamespace(n=n)
                pc.v_sb = vmap.pop(n)
                pc.u_sb = u_pool.tile([C, S], FP16, tag="u_sb")
                pc.g_ps = g_ps_pool.tile([C, C], FP, tag="g_ps")
                sm[n] = pc

                s2 = pend_s2.pop(0) if len(pend_s2) >= PIPE else []

                def tr_batch(j):
                    # 8 transposes -> one [C,1024] PSUM tile -> one DVE copy
                    t_ps = tr_ps_pool.tile([C, 1024], FP16, tag="t_ps")
                    for q in range(8):
                        k = 8 * j + q
                        nc.tensor.transpose(
                            t_ps[:, 128 * q : 128 * (q + 1)],
                            pc.v_sb[:, 128 * k : 128 * (k + 1)],
                            id_sb[:],
                        )
                    nc.vector.tensor_copy(
                        pc.u_sb[:, 1024 * j : 1024 * (j + 1)], t_ps[:]
                    )

                def mm1(j):
                    for q in range(8):
                        k = 8 * j + q
                        ck = pc.u_sb[:, 128 * k : 128 * (k + 1)]
                        nc.tensor.matmul(
                            pc.g_ps[:], ck, ck,
                            start=(k == 0), stop=(k == NCHUNK - 1),
                        )

                # stage-2 units land in the first half of the iteration so
                # their cast-copies finish well before the next problem's
                # units need the PSUM slots back
                def s2pop(k=1):
                    for _ in range(k):
                        if s2:
                            s2.pop(0)()

                tr_batch(0)
                s2pop(2)
                tr_batch(1)
                s2pop(2)
                if pend_at:
                    at_finish(sm.pop(pend_at.pop(0)))
                tr_batch(2)
                mm1(0)
                tr_batch(3)
                mm1(1)
                s2pop()   # the deferred DVE half-copy + second output DMA
                mm1(2)
                mm1(3)

                # softmax head: rmin (DVE) -> exp(rmin - G) (ACT) -> Z row
                # sum (DVE) -> 1/Z (DVE) -> nsc = numer * (gamma/Z) (GpSimd,
                # all-SBUF so the otherwise-idle Pool engine can own it);
                # the transpose + (+I) tail runs early next iteration so the
                # PE never waits on this chain.
                rmin = small_pool.tile([C, 1], FP, tag="rmin")
                nc.vector.tensor_reduce(rmin[:], pc.g_ps[:], axis=AX.X, op=OP.min)
                numer = small_pool.tile([C, C], FP16, tag="numer")
                zsum = small_pool.tile([C, 1], FP, tag="zsum")
                nc.scalar.activation(
                    numer[:], pc.g_ps[:], AF.Exp,
                    bias=rmin[:], scale=-1.0, accum_out=zsum[:],
                )
                zinv = small_pool.tile([C, 1], FP, tag="zinv")
                nc.vector.reciprocal(zinv[:], zsum[:])
                nsc = small_pool.tile([C, C], FP16, tag="nsc")
                nc.gpsimd.tensor_scalar(
                    nsc[:], numer[:], zinv[:], gam_sb[:],
                    op0=OP.mult, op1=OP.mult,
                )
                pc.nsc = nsc
                pend_at.append(n)
                pend_s2.append(
                    make_stage2_units(pc, drain=(n >= n_per_core - PIPE))
                )
            else:
                # drain: emit the ready stage-2 burst first so the PE works
                # while the last softmax chain finishes, then the tail
                s2 = pend_s2.pop(0) if pend_s2 else []
                if s2:
                    s2.pop(0)()
                    s2.pop(0)()
                if pend_at:
                    at_finish(sm.pop(pend_at.pop(0)))
                while s2:
                    s2.pop(0)()

    nc.compile()
    _CACHE[key] = nc
    return nc


def make_in_maps(x, gamma, n_per_core=N_PER_CORE):
    """Shard full inputs into per-core input maps (data-parallel over B*D)."""
    x = np.asarray(x, dtype=np.float32)
    gamma = np.asarray(gamma, dtype=np.float32).reshape(-1)
    gamma_b = np.full((C, 1), gamma[0], dtype=np.float32)
    ident = np.eye(C, dtype=np.float16)
    # v[n=(b,d)][c,s] = x[b,c,d,s] ; core i takes n in [i*npc, (i+1)*npc)
    xt = (
        x.reshape(B, C, D, S).transpose(0, 2, 1, 3).astype(np.float16)
    ).reshape(N_TOTAL, C, S)
    in_maps = []
    for i in range(N_CORES):
        xs = np.ascontiguousarray(xt[i * n_per_core : (i + 1) * n_per_core])
        in_maps.append({"xs": xs, "gamma_b": gamma_b, "ident": ident})
    return in_maps


def run_on_cores(x, gamma, trace=False, **kw):
    nc = build_program()
    in_maps = make_in_maps(x, gamma)
    res = run_bass_kernel_spmd(
        nc, in_maps, core_ids=list(range(N_CORES)), trace=trace, **kw
    )
    return res


def assemble_output(results):
    parts = [results[i]["out"] for i in range(N_CORES)]
    full = np.concatenate(parts, axis=0).astype(np.float32)  # (B*D, C, S)
    # reference returns a raw reinterpret of contiguous (B, D, C, H, W)
    return full.reshape(B, C, D, H, W)


def kernel(x, gamma):
    res = run_on_cores(x, gamma, trace=False)
    return assemble_output(res.results)
